# revision 3
# baseline (speedup 1.0000x reference)
"""Trainium2 Bass kernel for int4-grouped-quantized linear (GPTQ-style).

out[8192, 11008] = x[8192, 4096] @ dequant(qweight, qzeros, scales)

Sharding: column-parallel over out_features N across 8 NeuronCores.
Each core dequantizes its W shard [4096, 1376] on-chip, loads x already
transposed via X-bar DMA-transpose (keeping the PE free for matmuls), and
runs dense fp16 matmuls with fp32 PSUM accumulation.
"""

import sys

sys.path.insert(0, "/opt/trn_rl_repo")

from contextlib import ExitStack

import numpy as np

import concourse.bass as bass
from concourse import bacc
import concourse.tile as tile
from concourse import mybir
from concourse.bass_utils import run_bass_kernel_spmd

AOT = mybir.AluOpType
F16, I32, F32 = mybir.dt.float16, mybir.dt.int32, mybir.dt.float32

T, K, N = 8192, 4096, 11008
NCORES = 8
NS = N // NCORES  # 1376 out cols per core
CS = NS // 8  # 172 packed int32 cols per core
G = 32  # quant groups (group size 128 == one k-block)
KB = K // 128  # 32 k-blocks
TC = 256  # t rows per x-transpose chunk
NCH = T // TC  # 32 chunks
TSUB = TC // 128  # 2 output row-blocks per chunk
SEGS = [(0, 512), (512, 512), (1024, 352)]  # N segments (PSUM bank sized)


def _body(ctx, tc, xd, qwd, qzd, scd, outd, zscr):
    nc = tc.nc
    cpool = ctx.enter_context(tc.tile_pool(name="const", bufs=1))
    qpool = ctx.enter_context(tc.tile_pool(name="qwp", bufs=4))
    stpool = ctx.enter_context(tc.tile_pool(name="stage", bufs=2))
    wpool = ctx.enter_context(tc.tile_pool(name="w", bufs=KB))
    bcpool = ctx.enter_context(tc.tile_pool(name="bc", bufs=3))
    xtpool = ctx.enter_context(tc.tile_pool(name="xt", bufs=2))
    pspool = ctx.enter_context(tc.tile_pool(name="ps", bufs=2, space="PSUM"))
    opool = ctx.enter_context(tc.tile_pool(name="o", bufs=3))

    # ---- unpack zero-points: qz [G, CS] i32 -> z [G, NS] f16, park in DRAM ----
    qz_t = cpool.tile([G, CS], I32)
    nc.gpsimd.dma_start(qz_t[:], qzd)
    z_stage = cpool.tile([G, NS], I32)
    for j in range(8):
        nc.vector.tensor_scalar(
            z_stage[:, j::8], qz_t[:], 4 * j, 0xF,
            AOT.logical_shift_right, AOT.bitwise_and,
        )
    z_t = cpool.tile([G, NS], F16)
    nc.vector.tensor_copy(z_t[:], z_stage[:])
    nc.gpsimd.dma_start(zscr, z_t[:])

    # ---- dequantize W = (w4 - z) * s, one k-block (= one quant group) at a time ----
    w_tiles = []
    for b in range(KB):
        qw_t = qpool.tile([128, CS], I32)
        nc.gpsimd.dma_start(qw_t[:], qwd[b * 128 : (b + 1) * 128, :])
        w_stage = stpool.tile([128, NS], I32)
        for j in range(8):
            nc.vector.tensor_scalar(
                w_stage[:, j::8], qw_t[:], 4 * j, 0xF,
                AOT.logical_shift_right, AOT.bitwise_and,
            )
        w_t = wpool.tile([128, NS], F16)
        nc.scalar.copy(w_t[:], w_stage[:])
        # replicate this group's zero/scale row across 128 partitions via DMA
        z_bc = bcpool.tile([128, NS], F16, tag="zbc")
        nc.gpsimd.dma_start(z_bc[:], zscr[b : b + 1, :].partition_broadcast(128))
        s_bc = bcpool.tile([128, NS], F16, tag="sbc")
        nc.gpsimd.dma_start(s_bc[:], scd[b : b + 1, :].partition_broadcast(128))
        nc.vector.tensor_tensor(w_t[:], w_t[:], z_bc[:], AOT.subtract)
        nc.vector.tensor_tensor(w_t[:], w_t[:], s_bc[:], AOT.mult)
        w_tiles.append(w_t)

    # ---- x arrives transposed via X-bar DMA; PE does only matmuls ----
    for c in range(NCH):
        r0 = c * TC
        xts = []
        for b in range(KB):
            xt = xtpool.tile([128, TC], F16, tag=f"xt{b}")
            nc.sync.dma_start_transpose(
                xt[:], xd[r0 : r0 + TC, b * 128 : (b + 1) * 128]
            )
            xts.append(xt)
        for tsub in range(TSUB):
            ps = pspool.tile([128, NS], F32)
            for b in range(KB):
                st = xts[b][:, tsub * 128 : (tsub + 1) * 128]
                for off, sz in SEGS:
                    nc.tensor.matmul(
                        ps[:, off : off + sz],
                        st,
                        w_tiles[b][:, off : off + sz],
                        start=(b == 0),
                        stop=(b == KB - 1),
                    )
            ob = opool.tile([128, NS], F16)
            for off, sz in SEGS:
                nc.any.tensor_copy(ob[:, off : off + sz], ps[:, off : off + sz])
            ro = r0 + tsub * 128
            nc.gpsimd.dma_start(outd[ro : ro + 128, :], ob[:])


def build_kernel():
    nc = bacc.Bacc("TRN2", target_bir_lowering=False, debug=False)
    xd = nc.dram_tensor("x", [T, K], F16, kind="ExternalInput").ap()
    qwd = nc.dram_tensor("qw", [K, CS], I32, kind="ExternalInput").ap()
    qzd = nc.dram_tensor("qz", [G, CS], I32, kind="ExternalInput").ap()
    scd = nc.dram_tensor("sc", [G, NS], F16, kind="ExternalInput").ap()
    outd = nc.dram_tensor("out", [T, NS], F16, kind="ExternalOutput").ap()
    zscr = nc.dram_tensor("z_scratch", [G, NS], F16, kind="Internal").ap()
    with tile.TileContext(nc) as tc, ExitStack() as ctx:
        _body(ctx, tc, xd, qwd, qzd, scd, outd, zscr)
    nc.compile()
    return nc


_NC = None


def _get_nc():
    global _NC
    if _NC is None:
        _NC = build_kernel()
    return _NC


def make_in_maps(x, qweight, qzeros, scales):
    x = np.asarray(x, dtype=np.float16)
    qweight = np.asarray(qweight, dtype=np.int32)
    qzeros = np.asarray(qzeros, dtype=np.int32)
    scales = np.asarray(scales, dtype=np.float16)
    in_maps = []
    for c in range(NCORES):
        in_maps.append(
            {
                "x": x,
                "qw": np.ascontiguousarray(qweight[:, c * CS : (c + 1) * CS]),
                "qz": np.ascontiguousarray(qzeros[:, c * CS : (c + 1) * CS]),
                "sc": np.ascontiguousarray(scales[:, c * NS : (c + 1) * NS]),
            }
        )
    return in_maps


def run(in_maps, **kwargs):
    return run_bass_kernel_spmd(
        _get_nc(), in_maps, core_ids=list(range(NCORES)), **kwargs
    )


def kernel(x, qweight, qzeros, scales):
    res = run(make_in_maps(x, qweight, qzeros, scales))
    outs = [res.results[c]["out"] for c in range(NCORES)]
    return np.concatenate(outs, axis=1)


# revision 4
# speedup vs baseline: 1.5124x; 1.5124x over previous
"""Trainium2 Bass kernel for int4-grouped-quantized linear (GPTQ-style).

out[8192, 11008] = x[8192, 4096] @ dequant(qweight, qzeros, scales)

Sharding: column-parallel over out_features N across 8 NeuronCores.
Each core dequantizes its W shard [4096, 1376] on-chip, loads x already
transposed via X-bar DMA-transpose (keeping the PE free for matmuls), and
runs dense fp16 matmuls with fp32 PSUM accumulation.
"""

import sys

sys.path.insert(0, "/opt/trn_rl_repo")

from contextlib import ExitStack

import numpy as np

import concourse.bass as bass
from concourse import bacc
import concourse.tile as tile
from concourse import mybir
from concourse.bass_utils import run_bass_kernel_spmd

AOT = mybir.AluOpType
F16, I32, F32 = mybir.dt.float16, mybir.dt.int32, mybir.dt.float32

T, K, N = 8192, 4096, 11008
NCORES = 8
NS = N // NCORES  # 1376 out cols per core
CS = NS // 8  # 172 packed int32 cols per core
G = 32  # quant groups (group size 128 == one k-block)
KB = K // 128  # 32 k-blocks
TC = 256  # t rows per x-transpose chunk
NCH = T // TC  # 32 chunks
TSUB = TC // 128  # 2 output row-blocks per chunk
SEGS = [(0, 512), (512, 512), (1024, 352)]  # N segments (PSUM bank sized)


def _body(ctx, tc, xd, qwd, qzd, scd, outd, zscr):
    nc = tc.nc
    cpool = ctx.enter_context(tc.tile_pool(name="const", bufs=1))
    qpool = ctx.enter_context(tc.tile_pool(name="qwp", bufs=4))
    stpool = ctx.enter_context(tc.tile_pool(name="stage", bufs=2))
    wpool = ctx.enter_context(tc.tile_pool(name="w", bufs=KB))
    bcpool = ctx.enter_context(tc.tile_pool(name="bc", bufs=3))
    xtpool = ctx.enter_context(tc.tile_pool(name="xt", bufs=2))
    pspool = ctx.enter_context(tc.tile_pool(name="ps", bufs=2, space="PSUM"))
    opool = ctx.enter_context(tc.tile_pool(name="o", bufs=3))

    # ---- unpack zero-points: qz [G, CS] i32 -> z [G, NS] f16, park in DRAM ----
    qz_t = cpool.tile([G, CS], I32)
    nc.gpsimd.dma_start(qz_t[:], qzd)
    z_stage = cpool.tile([G, NS], I32)
    for j in range(8):
        nc.vector.tensor_scalar(
            z_stage[:, j::8], qz_t[:], 4 * j, 0xF,
            AOT.logical_shift_right, AOT.bitwise_and,
        )
    z_t = cpool.tile([G, NS], F16)
    nc.vector.tensor_copy(z_t[:], z_stage[:])
    nc.gpsimd.dma_start(zscr, z_t[:])

    # ---- dequantize W = (w4 - z) * s, one k-block (= one quant group) at a time ----
    w_tiles = []
    for b in range(KB):
        qw_t = qpool.tile([128, CS], I32)
        nc.gpsimd.dma_start(qw_t[:], qwd[b * 128 : (b + 1) * 128, :])
        w_stage = stpool.tile([128, NS], I32)
        for j in range(8):
            nc.vector.tensor_scalar(
                w_stage[:, j::8], qw_t[:], 4 * j, 0xF,
                AOT.logical_shift_right, AOT.bitwise_and,
            )
        w_t = wpool.tile([128, NS], F16)
        nc.scalar.copy(w_t[:], w_stage[:])
        # replicate this group's zero/scale row across 128 partitions via DMA
        z_bc = bcpool.tile([128, NS], F16, tag="zbc")
        nc.gpsimd.dma_start(z_bc[:], zscr[b : b + 1, :].partition_broadcast(128))
        s_bc = bcpool.tile([128, NS], F16, tag="sbc")
        nc.gpsimd.dma_start(s_bc[:], scd[b : b + 1, :].partition_broadcast(128))
        nc.vector.tensor_tensor(w_t[:], w_t[:], z_bc[:], AOT.subtract)
        nc.vector.tensor_tensor(w_t[:], w_t[:], s_bc[:], AOT.mult)
        w_tiles.append(w_t)

    # ---- x arrives transposed via X-bar DMA; PE does only matmuls ----
    # One transpose instruction per chunk: [TC, K] DRAM -> [128, KB, TC] SBUF,
    # xt[p, b, t] = x[r0 + t, b*128 + p].
    for c in range(NCH):
        r0 = c * TC
        xt = xtpool.tile([128, KB, TC], F16, tag="xt")
        nc.sync.dma_start_transpose(xt[:], xd[r0 : r0 + TC, :])
        for tsub in range(TSUB):
            ps = pspool.tile([128, NS], F32)
            for b in range(KB):
                st = xt[:, b, tsub * 128 : (tsub + 1) * 128]
                for off, sz in SEGS:
                    nc.tensor.matmul(
                        ps[:, off : off + sz],
                        st,
                        w_tiles[b][:, off : off + sz],
                        start=(b == 0),
                        stop=(b == KB - 1),
                    )
            ob = opool.tile([128, NS], F16)
            for off, sz in SEGS:
                nc.any.tensor_copy(ob[:, off : off + sz], ps[:, off : off + sz])
            ro = r0 + tsub * 128
            nc.gpsimd.dma_start(outd[ro : ro + 128, :], ob[:])


def build_kernel():
    nc = bacc.Bacc("TRN2", target_bir_lowering=False, debug=False)
    xd = nc.dram_tensor("x", [T, K], F16, kind="ExternalInput").ap()
    qwd = nc.dram_tensor("qw", [K, CS], I32, kind="ExternalInput").ap()
    qzd = nc.dram_tensor("qz", [G, CS], I32, kind="ExternalInput").ap()
    scd = nc.dram_tensor("sc", [G, NS], F16, kind="ExternalInput").ap()
    outd = nc.dram_tensor("out", [T, NS], F16, kind="ExternalOutput").ap()
    zscr = nc.dram_tensor("z_scratch", [G, NS], F16, kind="Internal").ap()
    with tile.TileContext(nc) as tc, ExitStack() as ctx:
        _body(ctx, tc, xd, qwd, qzd, scd, outd, zscr)
    nc.compile()
    return nc


_NC = None


def _get_nc():
    global _NC
    if _NC is None:
        _NC = build_kernel()
    return _NC


def make_in_maps(x, qweight, qzeros, scales):
    x = np.asarray(x, dtype=np.float16)
    qweight = np.asarray(qweight, dtype=np.int32)
    qzeros = np.asarray(qzeros, dtype=np.int32)
    scales = np.asarray(scales, dtype=np.float16)
    in_maps = []
    for c in range(NCORES):
        in_maps.append(
            {
                "x": x,
                "qw": np.ascontiguousarray(qweight[:, c * CS : (c + 1) * CS]),
                "qz": np.ascontiguousarray(qzeros[:, c * CS : (c + 1) * CS]),
                "sc": np.ascontiguousarray(scales[:, c * NS : (c + 1) * NS]),
            }
        )
    return in_maps


def run(in_maps, **kwargs):
    return run_bass_kernel_spmd(
        _get_nc(), in_maps, core_ids=list(range(NCORES)), **kwargs
    )


def kernel(x, qweight, qzeros, scales):
    res = run(make_in_maps(x, qweight, qzeros, scales))
    outs = [res.results[c]["out"] for c in range(NCORES)]
    return np.concatenate(outs, axis=1)
